# revision 32
# baseline (speedup 1.0000x reference)
"""SuperposedExpert (K TT-factorized FFN paths + holographic routing) on 8 trn2 cores.

Strategy: expert x data parallel. Core c handles path k = c % 4 for token half
c // 4. On-device per core:
  1. logits/softmax gating from bf16 tokens (tiny matmuls on PE), per n-chunk.
  2. TT expansion: W = G1 x_r G2 via rank-16 row-packed matmuls; PSUM drained
     (DVE/ACT split) to bounce tiles, then permuting SBUF->SBUF DMA scatters
     convert the Kronecker-mixed layout [(a,x),(b,y)] into the dense matmul
     layout [(a,b),(x,y)] directly into RESIDENT weight tiles (wb1, wb2).
     No DRAM roundtrip. W1 scatters issue on sync (HWDGE), W2 on gpsimd (SWDGE)
     to spread sequencer issue cost across idle engines.
  3. Dense bf16 FFN per 512-token chunk: hT = gelu(W1^T @ xT), oT = W2^T @ hT.
  4. Scale by gate[n] (partition-broadcast) and (1 + path_weight[d]), cast bf16,
     ReduceScatter(add) over the 4 cores sharing the token half, per chunk.
Host only reshapes/casts inputs and concatenates/transposes the output pieces.
"""

import numpy as np
import ml_dtypes

import concourse.bass as bass
import concourse.tile as tile
from concourse import bacc, mybir
from concourse.bass import ds, ts
from concourse.bass_utils import run_bass_kernel_spmd

BF16 = mybir.dt.bfloat16
F32 = mybir.dt.float32
AF = mybir.ActivationFunctionType

K = 4
D = 1024            # d_model
DFF = 4096          # d_ff
R = 16              # tt rank
IN1, IN2 = 32, 32   # d_model = IN1 * IN2   (a, b)
F1, F2 = 64, 64     # d_ff    = F1 * F2     (x, y)
NTOK = 2048
NCORES = 8
NHALF = NTOK // 2   # tokens per core
NCH = 512           # n-chunk (psum bank = 512 fp32)
NNCH = NHALF // NCH
GROUPS = [[0, 1, 2, 3], [4, 5, 6, 7]]


def _emit(nc, tc):
    # ---------------- I/O ----------------
    xT = nc.dram_tensor("xT", [D, NHALF], BF16, kind="ExternalInput")
    # TT cores packed for 4-way row-tiled rank-16 matmuls: row group i
    # (partitions 32i..32i+15) holds stationary chunk 4q+i / a replica of
    # the moving operand.
    g1pk = nc.dram_tensor("g1pk", [128, 512], BF16, kind="ExternalInput")
    g2pk = nc.dram_tensor("g2pk", [128, 2048], BF16, kind="ExternalInput")
    c1pk = nc.dram_tensor("c1pk", [128, 512], BF16, kind="ExternalInput")
    c2pk = nc.dram_tensor("c2pk", [128, 2048], BF16, kind="ExternalInput")
    pbT = nc.dram_tensor("pbT", [D, K], BF16, kind="ExternalInput")
    pw = nc.dram_tensor("pw", [128, 8], F32, kind="ExternalInput")
    sel = nc.dram_tensor("sel", [K, 1], F32, kind="ExternalInput")
    ones4 = nc.dram_tensor("ones4", [K, 1], F32, kind="ExternalInput")
    ones1 = nc.dram_tensor("ones1", [1, 128], F32, kind="ExternalInput")
    opiece = nc.dram_tensor("opiece", [D // K, NHALF], BF16, kind="ExternalOutput")

    # DRAM bounce buffers for the per-chunk ReduceScatter (bf16)
    cc_in = [nc.dram_tensor(f"cc_in{i}", [D, NCH], BF16) for i in range(NNCH)]
    cc_out = [nc.dram_tensor(f"cc_out{i}", [D // K, NCH], BF16) for i in range(NNCH)]
    # DRAM staging for the W permute: bounce tiles dumped verbatim (big
    # contiguous descriptors, fast completion), permuting happens on the
    # load side (DRAM src APs are unrestricted; SBUF dst partition-first).
    wdump = nc.dram_tensor("wdump", [32, 128, 2048], BF16)

    with (
        tc.tile_pool(name="big", bufs=1) as big,
        tc.tile_pool(name="small", bufs=1) as small,
        tc.tile_pool(name="xtp", bufs=1) as xtp,
        tc.tile_pool(name="gkp", bufs=2) as gkp,
        tc.tile_pool(name="bounce", bufs=2) as bounce,
        tc.tile_pool(name="obp", bufs=1) as obp,
        tc.tile_pool(name="pp", bufs=8, space="PSUM") as pp,
    ):
        # ---------------- resident weight tiles ----------------
        # W1 dense layout: [p=(a%4, b), s=d-chunk, f=(x,y)]
        wb1 = big.tile([128, 8, DFF], BF16, tag="wb1")
        # W2 dense layout: [p=(f1%2, f2), kc=f-chunk, d=(i1,i2)]
        wb2 = big.tile([128, 32, D], BF16, tag="wb2")
        ht = big.tile([128, 32, NCH], BF16, tag="ht")  # single buffer, serial reuse

        # ---------------- small loads ----------------
        pbt_sb = small.tile([128, 8, K], BF16, tag="pbt")
        nc.sync.dma_start(pbt_sb, pbT.ap().rearrange("(t p) k -> p t k", p=128))
        pw_sb = small.tile([128, 8], F32, tag="pw")
        nc.sync.dma_start(pw_sb, pw.ap())
        sel_sb = small.tile([K, 1], F32, tag="sel")
        nc.sync.dma_start(sel_sb, sel.ap())
        ones4_sb = small.tile([K, 1], F32, tag="ones4")
        nc.sync.dma_start(ones4_sb, ones4.ap())
        ones1_sb = small.tile([1, 128], F32, tag="ones1")
        nc.sync.dma_start(ones1_sb, ones1.ap())

        g1_sb = small.tile([128, 512], BF16, tag="g1")
        nc.sync.dma_start(g1_sb, g1pk.ap())
        g2_sb = small.tile([128, 2048], BF16, tag="g2")
        nc.sync.dma_start(g2_sb, g2pk.ap())
        c1_sb = small.tile([128, 512], BF16, tag="c1")
        nc.sync.dma_start(c1_sb, c1pk.ap())
        c2_sb = small.tile([128, 2048], BF16, tag="c2")
        nc.sync.dma_start(c2_sb, c2pk.ap())

        xt_view = xT.ap().rearrange("(t p) n -> p t n", p=128)
        xts = {}

        def load_xt(nch):
            xt_t = xtp.tile([128, 8, NCH], BF16, tag="xt", name=f"xt_{nch}")
            nc.sync.dma_start(xt_t, xt_view[:, :, ts(nch, NCH)])
            xts[nch] = xt_t

        load_xt(0)

        # ---------------- gating helper (per n-chunk) ----------------
        def gating(nch):
            xt_t = xts[nch]
            expl = gkp.tile([K, NCH], F32, tag="expl", name=f"expl_{nch}")
            lps = pp.tile([K, NCH], F32, tag="ps", name=f"lps_{nch}")
            for t in range(8):
                nc.tensor.matmul(
                    lps, pbt_sb[:, t], xt_t[:, t], start=(t == 0), stop=(t == 7)
                )
            nc.scalar.activation(expl, lps, AF.Exp)
            den = pp.tile([1, NCH], F32, tag="ps", name=f"den_{nch}")
            num = pp.tile([1, NCH], F32, tag="ps", name=f"num_{nch}")
            nc.tensor.matmul(den, ones4_sb, expl)
            nc.tensor.matmul(num, sel_sb, expl)
            gk = gkp.tile([1, NCH], F32, tag="gk", name=f"gk_{nch}")
            nc.vector.reciprocal(gk, den)
            nc.vector.tensor_mul(gk, num, gk)
            # broadcast gate row to 128 partitions via PE (DVE can't
            # partition-broadcast): gbc = ones1^T @ gk
            gbc = gkp.tile([128, NCH], BF16, tag="gbc", name=f"gbc_{nch}")
            gps = pp.tile([128, NCH], F32, tag="ps", name=f"gps_{nch}")
            nc.tensor.matmul(gps, ones1_sb, gk)
            nc.vector.tensor_copy(gbc, gps)
            return gbc

        gk0 = gating(0)

        # ------------- TT expansion: matmuls + drains + dumps + gathers ----
        # MMs emitted nq-outer/i-inner: the 4 row-group tiles run concurrently
        # on the PE. Each bounce tile: 4 drains (DVE/ACT split), a contiguous
        # dump to DRAM (fast completion frees the bounce slot), then permuting
        # gather-loads (SWDGE on the idle GpSimd queue) into resident wb1/wb2.
        def expansion(w, l_sb, r_sb):
            for q in range(4):
                eps = {}
                for nq in range(4):
                    for i in range(4):
                        e = pp.tile([128, NCH], F32, tag="ps",
                                    name=f"pe{w}_{q}_{nq}_{i}")
                        nc.tensor.matmul(
                            e, l_sb[ds(32 * i, R), ts(q, 128)],
                            r_sb[ds(32 * i, R), ts(nq, NCH)],
                            tile_position=(32 * i, 0),
                        )
                        eps[(nq, i)] = e
                for i in range(4):
                    mt = 4 * q + i
                    bt = bounce.tile([128, 2048], BF16, tag="bt",
                                     name=f"bt{w}_{q}_{i}")
                    for nq in range(4):
                        if (nq + i) % 2 == 0:
                            nc.vector.tensor_copy(bt[:, ts(nq, NCH)], eps[(nq, i)])
                        else:
                            nc.scalar.activation(
                                bt[:, ts(nq, NCH)], eps[(nq, i)], AF.Copy)
                    slot = (w - 1) * 16 + mt
                    eng = nc.sync if i % 2 == 0 else nc.scalar
                    eng.dma_start(wdump[ds(slot, 1)].squeeze(), bt)
                    if w == 1:
                        for a2 in range(2):
                            sv = mt // 2
                            s_ap = wdump[ds(slot, 1)].squeeze().rearrange(
                                "(a2 x) (b y) -> a2 b x y", a2=2, y=64
                            )[ds(a2, 1)].squeeze()
                            d_ap = wb1[ds(64 * (mt % 2) + 32 * a2, 32), sv] \
                                .rearrange("b (x y) -> b x y", y=64)
                            idx = 2 * mt + a2
                            geng = nc.sync if idx % 4 == 3 else nc.gpsimd
                            geng.dma_start(d_ap, s_ap)
                    else:
                        for fl in range(4):
                            f1 = 4 * mt + fl
                            kcv, fhv = f1 // 2, f1 % 2
                            s_ap = wdump[ds(slot, 1)].squeeze().rearrange(
                                "(fl i1) (f2 i2) -> fl f2 i1 i2", fl=4, i2=32
                            )[ds(fl, 1)].squeeze()
                            d_ap = wb2[ds(64 * fhv, 64), kcv].rearrange(
                                "f2 (i1 i2) -> f2 i1 i2", i2=32)
                            geng = (nc.gpsimd, nc.sync, nc.gpsimd, nc.scalar)[fl]
                            geng.dma_start(d_ap, s_ap)

        expansion(1, g1_sb, g2_sb)
        expansion(2, c1_sb, c2_sb)

        # ---------------- ffn1 ----------------
        def ffn1(nch):
            xt_t = xts[nch]
            for grp in range(4):
                ps_l1 = [
                    pp.tile([128, NCH], F32, tag="ps", name=f"ps1_{nch}_{grp}_{i}")
                    for i in range(8)
                ]
                for s in range(8):
                    for j in range(8):
                        m = grp * 8 + j
                        nc.tensor.matmul(
                            ps_l1[j], wb1[:, s, ts(m, 128)], xt_t[:, s],
                            start=(s == 0), stop=(s == 7),
                        )
                for j in range(8):
                    nc.scalar.activation(
                        ht[:, grp * 8 + j], ps_l1[j], AF.Gelu_apprx_tanh
                    )

        ffn1(0)

        # ---------------- ffn2 + scale + RS per chunk ----------------
        def ffn2(nch, gbc):
            ps_l = [
                pp.tile([128, NCH], F32, tag="ps", name=f"ps2_{nch}_{i}")
                for i in range(8)
            ]
            for kc in range(32):
                for m2 in range(8):
                    nc.tensor.matmul(
                        ps_l[m2], wb2[:, kc, ts(m2, 128)], ht[:, kc],
                        start=(kc == 0), stop=(kc == 31),
                    )
            ob = obp.tile([128, 8, NCH], BF16, tag="ob", name=f"ob_{nch}")
            for m2 in range(8):
                nc.vector.tensor_mul(ob[:, m2], ps_l[m2], gbc)
                nc.vector.tensor_scalar_mul(ob[:, m2], ob[:, m2], pw_sb[:, ds(m2, 1)])
            nc.scalar.dma_start(
                cc_in[nch].ap().rearrange("(m2 p) n -> p m2 n", p=128), ob
            )
            nc.gpsimd.collective_compute(
                "ReduceScatter",
                mybir.AluOpType.add,
                replica_groups=GROUPS,
                ins=[cc_in[nch][:]],
                outs=[cc_out[nch][:]],
            )
            nc.scalar.dma_start(opiece[:, ts(nch, NCH)], cc_out[nch][:])

        ffn2(0, gk0)

        for nch in range(1, NNCH):
            load_xt(nch)
            gbc = gating(nch)
            ffn1(nch)
            ffn2(nch, gbc)


def build(verbose=False):
    nc = bacc.Bacc("TRN2", target_bir_lowering=False, debug=False, num_devices=NCORES)
    with tile.TileContext(nc) as tc:
        _emit(nc, tc)
    nc.compile()
    return nc


def make_in_maps(inputs):
    tokens = inputs["tokens"]
    bf = ml_dtypes.bfloat16
    in_maps = []
    for c in range(NCORES):
        half, k = c // 4, c % 4
        xt = np.ascontiguousarray(
            tokens[half * NHALF:(half + 1) * NHALF].T
        ).astype(bf)
        g1t = inputs["ffn1_core1"][k].transpose(2, 0, 1).reshape(R, IN1 * F1)
        g2 = inputs["ffn1_core2"][k].reshape(R, IN2 * F2)
        c1t = inputs["ffn2_core1"][k].transpose(2, 0, 1).reshape(R, F1 * IN1)
        c2 = inputs["ffn2_core2"][k].reshape(R, F2 * IN2)

        def pack_lhs(m):  # [R, 2048] -> [128, 512]: row group i gets chunk 4q+i
            out = np.zeros((128, 512), np.float32)
            for q in range(4):
                for i in range(4):
                    out[32 * i:32 * i + R, 128 * q:128 * (q + 1)] = \
                        m[:, 128 * (4 * q + i):128 * (4 * q + i + 1)]
            return out

        def pack_rhs(m):  # [R, 2048] -> [128, 2048]: replicate per row group
            out = np.zeros((128, 2048), np.float32)
            for i in range(4):
                out[32 * i:32 * i + R] = m
            return out
        pbt = np.ascontiguousarray(inputs["path_bases"].T).astype(bf)
        pwk = np.ascontiguousarray(
            (1.0 + inputs["path_weights"][k]).reshape(8, 128).T
        ).astype(np.float32)
        selk = np.zeros((K, 1), np.float32)
        selk[k, 0] = 1.0
        in_maps.append({
            "xT": xt,
            "g1pk": pack_lhs(g1t).astype(bf), "g2pk": pack_rhs(g2).astype(bf),
            "c1pk": pack_lhs(c1t).astype(bf), "c2pk": pack_rhs(c2).astype(bf),
            "pbT": pbt, "pw": pwk, "sel": selk,
            "ones4": np.ones((K, 1), np.float32),
            "ones1": np.ones((1, 128), np.float32),
        })
    return in_maps


def assemble(results):
    out = np.empty((NTOK, D), np.float32)
    for c in range(NCORES):
        half, k = c // 4, c % 4
        piece = results[c]["opiece"]  # [256 d-slice, 1024 tokens] bf16
        out[half * NHALF:(half + 1) * NHALF, k * 256:(k + 1) * 256] = \
            piece.T.astype(np.float32)
    return out


_NC = None


def run(inputs, trace=False):
    global _NC
    if _NC is None:
        _NC = build()
    res = run_bass_kernel_spmd(
        _NC, make_in_maps(inputs), core_ids=list(range(NCORES)), trace=trace
    )
    return assemble(res.results), res


def kernel(**inputs):
    out, _ = run(inputs)
    return out


# revision 35
# speedup vs baseline: 1.0248x; 1.0248x over previous
"""SuperposedExpert (K TT-factorized FFN paths + holographic routing) on 8 trn2 cores.

Strategy: expert x data parallel. Core c handles path k = c % 4 for token half
c // 4. On-device per core:
  1. logits/softmax gating from bf16 tokens (tiny matmuls on PE), per n-chunk.
  2. TT expansion: W = G1 x_r G2 via rank-16 row-packed matmuls; PSUM drained
     (DVE/ACT split) to bounce tiles, then permuting SBUF->SBUF DMA scatters
     convert the Kronecker-mixed layout [(a,x),(b,y)] into the dense matmul
     layout [(a,b),(x,y)] directly into RESIDENT weight tiles (wb1, wb2).
     No DRAM roundtrip. W1 scatters issue on sync (HWDGE), W2 on gpsimd (SWDGE)
     to spread sequencer issue cost across idle engines.
  3. Dense bf16 FFN per 512-token chunk: hT = gelu(W1^T @ xT), oT = W2^T @ hT.
  4. Scale by gate[n] (partition-broadcast) and (1 + path_weight[d]), cast bf16,
     ReduceScatter(add) over the 4 cores sharing the token half, per chunk.
Host only reshapes/casts inputs and concatenates/transposes the output pieces.
"""

import numpy as np
import ml_dtypes

import concourse.bass as bass
import concourse.tile as tile
from concourse import bacc, mybir
from concourse.bass import ds, ts
from concourse.bass_utils import run_bass_kernel_spmd

BF16 = mybir.dt.bfloat16
F32 = mybir.dt.float32
AF = mybir.ActivationFunctionType

K = 4
D = 1024            # d_model
DFF = 4096          # d_ff
R = 16              # tt rank
IN1, IN2 = 32, 32   # d_model = IN1 * IN2   (a, b)
F1, F2 = 64, 64     # d_ff    = F1 * F2     (x, y)
NTOK = 2048
NCORES = 8
NHALF = NTOK // 2   # tokens per core
NCH = 512           # n-chunk (psum bank = 512 fp32)
NNCH = NHALF // NCH
GROUPS = [[0, 1, 2, 3], [4, 5, 6, 7]]


def _emit(nc, tc):
    # ---------------- I/O ----------------
    xT = nc.dram_tensor("xT", [D, NHALF], BF16, kind="ExternalInput")
    # TT cores packed for 4-way row-tiled rank-16 matmuls: row group i
    # (partitions 32i..32i+15) holds stationary chunk 4q+i / a replica of
    # the moving operand.
    g1pk = nc.dram_tensor("g1pk", [128, 512], BF16, kind="ExternalInput")
    g2pk = nc.dram_tensor("g2pk", [128, 2048], BF16, kind="ExternalInput")
    c1pk = nc.dram_tensor("c1pk", [128, 512], BF16, kind="ExternalInput")
    c2pk = nc.dram_tensor("c2pk", [128, 2048], BF16, kind="ExternalInput")
    pbT = nc.dram_tensor("pbT", [D, K], BF16, kind="ExternalInput")
    pw = nc.dram_tensor("pw", [128, 8], F32, kind="ExternalInput")
    sel = nc.dram_tensor("sel", [K, 1], F32, kind="ExternalInput")
    ones4 = nc.dram_tensor("ones4", [K, 1], F32, kind="ExternalInput")
    ones1 = nc.dram_tensor("ones1", [1, 128], F32, kind="ExternalInput")
    opiece = nc.dram_tensor("opiece", [D // K, NHALF], BF16, kind="ExternalOutput")

    # DRAM bounce buffers for the per-chunk ReduceScatter (bf16)
    cc_in = [nc.dram_tensor(f"cc_in{i}", [D, NCH], BF16) for i in range(NNCH)]
    cc_out = [nc.dram_tensor(f"cc_out{i}", [D // K, NCH], BF16) for i in range(NNCH)]
    # DRAM staging for the W permute: bounce tiles dumped verbatim (big
    # contiguous descriptors, fast completion), permuting happens on the
    # load side (DRAM src APs are unrestricted; SBUF dst partition-first).
    wdump = nc.dram_tensor("wdump", [32, 128, 2048], BF16)

    with (
        tc.tile_pool(name="big", bufs=1) as big,
        tc.tile_pool(name="small", bufs=1) as small,
        tc.tile_pool(name="xtp", bufs=1) as xtp,
        tc.tile_pool(name="gkp", bufs=2) as gkp,
        tc.tile_pool(name="bounce", bufs=2) as bounce,
        tc.tile_pool(name="obp", bufs=1) as obp,
        tc.tile_pool(name="pp", bufs=8, space="PSUM") as pp,
    ):
        # ---------------- resident weight tiles ----------------
        # W1 dense layout: [p=(a%4, b), s=d-chunk, f=(x,y)]
        wb1 = big.tile([128, 8, DFF], BF16, tag="wb1")
        # W2 dense layout: [p=(f1%2, f2), kc=f-chunk, d=(i1,i2)]
        wb2 = big.tile([128, 32, D], BF16, tag="wb2")
        ht = big.tile([128, 32, NCH], BF16, tag="ht")  # single buffer, serial reuse

        # ---------------- small loads ----------------
        pbt_sb = small.tile([128, 8, K], BF16, tag="pbt")
        nc.sync.dma_start(pbt_sb, pbT.ap().rearrange("(t p) k -> p t k", p=128))
        pw_sb = small.tile([128, 8], F32, tag="pw")
        nc.sync.dma_start(pw_sb, pw.ap())
        sel_sb = small.tile([K, 1], F32, tag="sel")
        nc.sync.dma_start(sel_sb, sel.ap())
        ones4_sb = small.tile([K, 1], F32, tag="ones4")
        nc.sync.dma_start(ones4_sb, ones4.ap())
        ones1_sb = small.tile([1, 128], F32, tag="ones1")
        nc.sync.dma_start(ones1_sb, ones1.ap())

        g1_sb = small.tile([128, 512], BF16, tag="g1")
        nc.sync.dma_start(g1_sb, g1pk.ap())
        g2_sb = small.tile([128, 2048], BF16, tag="g2")
        nc.sync.dma_start(g2_sb, g2pk.ap())
        c1_sb = small.tile([128, 512], BF16, tag="c1")
        nc.sync.dma_start(c1_sb, c1pk.ap())
        c2_sb = small.tile([128, 2048], BF16, tag="c2")
        nc.sync.dma_start(c2_sb, c2pk.ap())

        xt_view = xT.ap().rearrange("(t p) n -> p t n", p=128)
        xts = {}

        def load_xt(nch):
            xt_t = xtp.tile([128, 8, NCH], BF16, tag="xt", name=f"xt_{nch}")
            nc.sync.dma_start(xt_t, xt_view[:, :, ts(nch, NCH)])
            xts[nch] = xt_t

        load_xt(0)

        # ---------------- gating helper (per n-chunk) ----------------
        def gating(nch):
            xt_t = xts[nch]
            expl = gkp.tile([K, NCH], F32, tag="expl", name=f"expl_{nch}")
            lps = pp.tile([K, NCH], F32, tag="ps", name=f"lps_{nch}")
            for t in range(8):
                nc.tensor.matmul(
                    lps, pbt_sb[:, t], xt_t[:, t], start=(t == 0), stop=(t == 7)
                )
            nc.scalar.activation(expl, lps, AF.Exp)
            den = pp.tile([1, NCH], F32, tag="ps", name=f"den_{nch}")
            num = pp.tile([1, NCH], F32, tag="ps", name=f"num_{nch}")
            nc.tensor.matmul(den, ones4_sb, expl)
            nc.tensor.matmul(num, sel_sb, expl)
            gk = gkp.tile([1, NCH], F32, tag="gk", name=f"gk_{nch}")
            nc.vector.reciprocal(gk, den)
            nc.vector.tensor_mul(gk, num, gk)
            # broadcast gate row to 128 partitions via PE (DVE can't
            # partition-broadcast): gbc = ones1^T @ gk
            gbc = gkp.tile([128, NCH], BF16, tag="gbc", name=f"gbc_{nch}")
            gps = pp.tile([128, NCH], F32, tag="ps", name=f"gps_{nch}")
            nc.tensor.matmul(gps, ones1_sb, gk)
            nc.vector.tensor_copy(gbc, gps)
            return gbc

        gk0 = gating(0)

        # ------------- TT expansion: matmuls + drains + dumps + gathers ----
        # MMs emitted nq-outer/i-inner: the 4 row-group tiles run concurrently
        # on the PE. Each bounce tile: 4 drains (DVE/ACT split), a contiguous
        # dump to DRAM (fast completion frees the bounce slot), then permuting
        # gather-loads (SWDGE on the idle GpSimd queue) into resident wb1/wb2.
        def expansion(w, l_sb, r_sb):
            for q in range(4):
                eps = {}
                for nq in range(4):
                    for i in range(4):
                        e = pp.tile([128, NCH], F32, tag="ps",
                                    name=f"pe{w}_{q}_{nq}_{i}")
                        nc.tensor.matmul(
                            e, l_sb[ds(32 * i, R), ts(q, 128)],
                            r_sb[ds(32 * i, R), ts(nq, NCH)],
                            tile_position=(32 * i, 0),
                        )
                        eps[(nq, i)] = e
                for i in range(4):
                    mt = 4 * q + i
                    bt = bounce.tile([128, 2048], BF16, tag="bt",
                                     name=f"bt{w}_{q}_{i}")
                    for nq in range(4):
                        if (nq + i) % 2 == 0:
                            nc.vector.tensor_copy(bt[:, ts(nq, NCH)], eps[(nq, i)])
                        else:
                            nc.scalar.activation(
                                bt[:, ts(nq, NCH)], eps[(nq, i)], AF.Copy)
                    slot = (w - 1) * 16 + mt
                    eng = nc.sync if i % 2 == 0 else nc.scalar
                    eng.dma_start(wdump[ds(slot, 1)].squeeze(), bt)

        # Gathers emitted AFTER the expansion's dumps so queue FIFOs never
        # block a dump behind a gather's dependency wait. W1 gathers stay on
        # GpSimd (fast SWDGE, first in its FIFO = ffn1 critical path); W2
        # gathers split GpSimd/Sync (sync's dumps are all issued by then).
        def gathers1():
            for mt in range(16):
                sv = mt // 2
                for a2 in range(2):
                    s_ap = wdump[ds(mt, 1)].squeeze().rearrange(
                        "(a2 x) (b y) -> a2 b x y", a2=2, y=64
                    )[ds(a2, 1)].squeeze()
                    d_ap = wb1[ds(64 * (mt % 2) + 32 * a2, 32), sv] \
                        .rearrange("b (x y) -> b x y", y=64)
                    nc.gpsimd.dma_start(d_ap, s_ap)

        def gathers2():
            for mt in range(16):
                for fl in range(4):
                    f1 = 4 * mt + fl
                    kcv, fhv = f1 // 2, f1 % 2
                    s_ap = wdump[ds(16 + mt, 1)].squeeze().rearrange(
                        "(fl i1) (f2 i2) -> fl f2 i1 i2", fl=4, i2=32
                    )[ds(fl, 1)].squeeze()
                    d_ap = wb2[ds(64 * fhv, 64), kcv].rearrange(
                        "f2 (i1 i2) -> f2 i1 i2", i2=32)
                    eng = nc.gpsimd if fl % 2 == 0 else nc.sync
                    eng.dma_start(d_ap, s_ap)

        expansion(1, g1_sb, g2_sb)
        gathers1()
        expansion(2, c1_sb, c2_sb)
        gathers2()

        # ---------------- ffn1 ----------------
        def ffn1(nch):
            xt_t = xts[nch]
            for grp in range(4):
                ps_l1 = [
                    pp.tile([128, NCH], F32, tag="ps", name=f"ps1_{nch}_{grp}_{i}")
                    for i in range(8)
                ]
                for s in range(8):
                    for j in range(8):
                        m = grp * 8 + j
                        nc.tensor.matmul(
                            ps_l1[j], wb1[:, s, ts(m, 128)], xt_t[:, s],
                            start=(s == 0), stop=(s == 7),
                        )
                for j in range(8):
                    nc.scalar.activation(
                        ht[:, grp * 8 + j], ps_l1[j], AF.Gelu_apprx_tanh
                    )

        ffn1(0)

        # ---------------- ffn2 + scale + RS per chunk ----------------
        def ffn2(nch, gbc):
            ps_l = [
                pp.tile([128, NCH], F32, tag="ps", name=f"ps2_{nch}_{i}")
                for i in range(8)
            ]
            for kc in range(32):
                for m2 in range(8):
                    nc.tensor.matmul(
                        ps_l[m2], wb2[:, kc, ts(m2, 128)], ht[:, kc],
                        start=(kc == 0), stop=(kc == 31),
                    )
            ob = obp.tile([128, 8, NCH], BF16, tag="ob", name=f"ob_{nch}")
            for m2 in range(8):
                nc.vector.tensor_mul(ob[:, m2], ps_l[m2], gbc)
                nc.vector.tensor_scalar_mul(ob[:, m2], ob[:, m2], pw_sb[:, ds(m2, 1)])
            nc.scalar.dma_start(
                cc_in[nch].ap().rearrange("(m2 p) n -> p m2 n", p=128), ob
            )
            nc.gpsimd.collective_compute(
                "ReduceScatter",
                mybir.AluOpType.add,
                replica_groups=GROUPS,
                ins=[cc_in[nch][:]],
                outs=[cc_out[nch][:]],
            )
            nc.scalar.dma_start(opiece[:, ts(nch, NCH)], cc_out[nch][:])

        ffn2(0, gk0)

        for nch in range(1, NNCH):
            load_xt(nch)
            gbc = gating(nch)
            ffn1(nch)
            ffn2(nch, gbc)


def build(verbose=False):
    nc = bacc.Bacc("TRN2", target_bir_lowering=False, debug=False, num_devices=NCORES)
    with tile.TileContext(nc) as tc:
        _emit(nc, tc)
    nc.compile()
    return nc


def make_in_maps(inputs):
    tokens = inputs["tokens"]
    bf = ml_dtypes.bfloat16
    in_maps = []
    for c in range(NCORES):
        half, k = c // 4, c % 4
        xt = np.ascontiguousarray(
            tokens[half * NHALF:(half + 1) * NHALF].T
        ).astype(bf)
        g1t = inputs["ffn1_core1"][k].transpose(2, 0, 1).reshape(R, IN1 * F1)
        g2 = inputs["ffn1_core2"][k].reshape(R, IN2 * F2)
        c1t = inputs["ffn2_core1"][k].transpose(2, 0, 1).reshape(R, F1 * IN1)
        c2 = inputs["ffn2_core2"][k].reshape(R, F2 * IN2)

        def pack_lhs(m):  # [R, 2048] -> [128, 512]: row group i gets chunk 4q+i
            out = np.zeros((128, 512), np.float32)
            for q in range(4):
                for i in range(4):
                    out[32 * i:32 * i + R, 128 * q:128 * (q + 1)] = \
                        m[:, 128 * (4 * q + i):128 * (4 * q + i + 1)]
            return out

        def pack_rhs(m):  # [R, 2048] -> [128, 2048]: replicate per row group
            out = np.zeros((128, 2048), np.float32)
            for i in range(4):
                out[32 * i:32 * i + R] = m
            return out
        pbt = np.ascontiguousarray(inputs["path_bases"].T).astype(bf)
        pwk = np.ascontiguousarray(
            (1.0 + inputs["path_weights"][k]).reshape(8, 128).T
        ).astype(np.float32)
        selk = np.zeros((K, 1), np.float32)
        selk[k, 0] = 1.0
        in_maps.append({
            "xT": xt,
            "g1pk": pack_lhs(g1t).astype(bf), "g2pk": pack_rhs(g2).astype(bf),
            "c1pk": pack_lhs(c1t).astype(bf), "c2pk": pack_rhs(c2).astype(bf),
            "pbT": pbt, "pw": pwk, "sel": selk,
            "ones4": np.ones((K, 1), np.float32),
            "ones1": np.ones((1, 128), np.float32),
        })
    return in_maps


def assemble(results):
    out = np.empty((NTOK, D), np.float32)
    for c in range(NCORES):
        half, k = c // 4, c % 4
        piece = results[c]["opiece"]  # [256 d-slice, 1024 tokens] bf16
        out[half * NHALF:(half + 1) * NHALF, k * 256:(k + 1) * 256] = \
            piece.T.astype(np.float32)
    return out


_NC = None


def run(inputs, trace=False):
    global _NC
    if _NC is None:
        _NC = build()
    res = run_bass_kernel_spmd(
        _NC, make_in_maps(inputs), core_ids=list(range(NCORES)), trace=trace
    )
    return assemble(res.results), res


def kernel(**inputs):
    out, _ = run(inputs)
    return out


# revision 36
# speedup vs baseline: 1.0277x; 1.0027x over previous
"""SuperposedExpert (K TT-factorized FFN paths + holographic routing) on 8 trn2 cores.

Strategy: expert x data parallel. Core c handles path k = c % 4 for token half
c // 4. On-device per core:
  1. logits/softmax gating from bf16 tokens (tiny matmuls on PE), per n-chunk.
  2. TT expansion: W = G1 x_r G2 via rank-16 row-packed matmuls; PSUM drained
     (DVE/ACT split) to bounce tiles, then permuting SBUF->SBUF DMA scatters
     convert the Kronecker-mixed layout [(a,x),(b,y)] into the dense matmul
     layout [(a,b),(x,y)] directly into RESIDENT weight tiles (wb1, wb2).
     No DRAM roundtrip. W1 scatters issue on sync (HWDGE), W2 on gpsimd (SWDGE)
     to spread sequencer issue cost across idle engines.
  3. Dense bf16 FFN per 512-token chunk: hT = gelu(W1^T @ xT), oT = W2^T @ hT.
  4. Scale by gate[n] (partition-broadcast) and (1 + path_weight[d]), cast bf16,
     ReduceScatter(add) over the 4 cores sharing the token half, per chunk.
Host only reshapes/casts inputs and concatenates/transposes the output pieces.
"""

import numpy as np
import ml_dtypes

import concourse.bass as bass
import concourse.tile as tile
from concourse import bacc, mybir
from concourse.bass import ds, ts
from concourse.bass_utils import run_bass_kernel_spmd

BF16 = mybir.dt.bfloat16
F32 = mybir.dt.float32
AF = mybir.ActivationFunctionType

K = 4
D = 1024            # d_model
DFF = 4096          # d_ff
R = 16              # tt rank
IN1, IN2 = 32, 32   # d_model = IN1 * IN2   (a, b)
F1, F2 = 64, 64     # d_ff    = F1 * F2     (x, y)
NTOK = 2048
NCORES = 8
NHALF = NTOK // 2   # tokens per core
NCH = 512           # n-chunk (psum bank = 512 fp32)
NNCH = NHALF // NCH
GROUPS = [[0, 1, 2, 3], [4, 5, 6, 7]]


def _emit(nc, tc):
    # ---------------- I/O ----------------
    xT = nc.dram_tensor("xT", [D, NHALF], BF16, kind="ExternalInput")
    # TT cores packed for 4-way row-tiled rank-16 matmuls: row group i
    # (partitions 32i..32i+15) holds stationary chunk 4q+i / a replica of
    # the moving operand.
    g1pk = nc.dram_tensor("g1pk", [128, 512], BF16, kind="ExternalInput")
    g2pk = nc.dram_tensor("g2pk", [128, 2048], BF16, kind="ExternalInput")
    c1pk = nc.dram_tensor("c1pk", [128, 512], BF16, kind="ExternalInput")
    c2pk = nc.dram_tensor("c2pk", [128, 2048], BF16, kind="ExternalInput")
    pbT = nc.dram_tensor("pbT", [D, K], BF16, kind="ExternalInput")
    pw = nc.dram_tensor("pw", [128, 8], F32, kind="ExternalInput")
    sel = nc.dram_tensor("sel", [K, 1], F32, kind="ExternalInput")
    ones4 = nc.dram_tensor("ones4", [K, 1], F32, kind="ExternalInput")
    ones1 = nc.dram_tensor("ones1", [1, 128], F32, kind="ExternalInput")
    opiece = nc.dram_tensor("opiece", [D // K, NHALF], BF16, kind="ExternalOutput")

    # DRAM bounce buffers for the per-chunk ReduceScatter (bf16)
    cc_in = [nc.dram_tensor(f"cc_in{i}", [D, NCH], BF16) for i in range(NNCH)]
    cc_out = [nc.dram_tensor(f"cc_out{i}", [D // K, NCH], BF16) for i in range(NNCH)]
    # DRAM staging for the W permute: bounce tiles dumped verbatim (big
    # contiguous descriptors, fast completion), permuting happens on the
    # load side (DRAM src APs are unrestricted; SBUF dst partition-first).
    wdump = nc.dram_tensor("wdump", [32, 128, 2048], BF16)

    with (
        tc.tile_pool(name="big", bufs=1) as big,
        tc.tile_pool(name="small", bufs=1) as small,
        tc.tile_pool(name="xtp", bufs=1) as xtp,
        tc.tile_pool(name="gkp", bufs=2) as gkp,
        tc.tile_pool(name="bounce", bufs=2) as bounce,
        tc.tile_pool(name="obp", bufs=1) as obp,
        tc.tile_pool(name="pp", bufs=8, space="PSUM") as pp,
    ):
        # ---------------- resident weight tiles ----------------
        # W1 dense layout: [p=(a%4, b), s=d-chunk, f=(x,y)]
        wb1 = big.tile([128, 8, DFF], BF16, tag="wb1")
        # W2 dense layout: [p=(f1%2, f2), kc=f-chunk, d=(i1,i2)]
        wb2 = big.tile([128, 32, D], BF16, tag="wb2")
        ht = big.tile([128, 32, NCH], BF16, tag="ht")  # single buffer, serial reuse

        # ---------------- small loads ----------------
        pbt_sb = small.tile([128, 8, K], BF16, tag="pbt")
        nc.sync.dma_start(pbt_sb, pbT.ap().rearrange("(t p) k -> p t k", p=128))
        pw_sb = small.tile([128, 8], F32, tag="pw")
        nc.sync.dma_start(pw_sb, pw.ap())
        sel_sb = small.tile([K, 1], F32, tag="sel")
        nc.sync.dma_start(sel_sb, sel.ap())
        ones4_sb = small.tile([K, 1], F32, tag="ones4")
        nc.sync.dma_start(ones4_sb, ones4.ap())
        ones1_sb = small.tile([1, 128], F32, tag="ones1")
        nc.sync.dma_start(ones1_sb, ones1.ap())

        g1_sb = small.tile([128, 512], BF16, tag="g1")
        nc.sync.dma_start(g1_sb, g1pk.ap())
        g2_sb = small.tile([128, 2048], BF16, tag="g2")
        nc.sync.dma_start(g2_sb, g2pk.ap())
        c1_sb = small.tile([128, 512], BF16, tag="c1")
        nc.sync.dma_start(c1_sb, c1pk.ap())
        c2_sb = small.tile([128, 2048], BF16, tag="c2")
        nc.sync.dma_start(c2_sb, c2pk.ap())

        xt_view = xT.ap().rearrange("(t p) n -> p t n", p=128)
        xts = {}

        def load_xt(nch):
            xt_t = xtp.tile([128, 8, NCH], BF16, tag="xt", name=f"xt_{nch}")
            nc.sync.dma_start(xt_t, xt_view[:, :, ts(nch, NCH)])
            xts[nch] = xt_t

        load_xt(0)

        # ---------------- gating helper (per n-chunk) ----------------
        def gating(nch):
            xt_t = xts[nch]
            expl = gkp.tile([K, NCH], F32, tag="expl", name=f"expl_{nch}")
            lps = pp.tile([K, NCH], F32, tag="ps", name=f"lps_{nch}")
            for t in range(8):
                nc.tensor.matmul(
                    lps, pbt_sb[:, t], xt_t[:, t], start=(t == 0), stop=(t == 7)
                )
            nc.scalar.activation(expl, lps, AF.Exp)
            den = pp.tile([1, NCH], F32, tag="ps", name=f"den_{nch}")
            num = pp.tile([1, NCH], F32, tag="ps", name=f"num_{nch}")
            nc.tensor.matmul(den, ones4_sb, expl)
            nc.tensor.matmul(num, sel_sb, expl)
            gk = gkp.tile([1, NCH], F32, tag="gk", name=f"gk_{nch}")
            nc.vector.reciprocal(gk, den)
            nc.vector.tensor_mul(gk, num, gk)
            # broadcast gate row to 128 partitions via PE (DVE can't
            # partition-broadcast): gbc = ones1^T @ gk
            gbc = gkp.tile([128, NCH], BF16, tag="gbc", name=f"gbc_{nch}")
            gps = pp.tile([128, NCH], F32, tag="ps", name=f"gps_{nch}")
            nc.tensor.matmul(gps, ones1_sb, gk)
            nc.vector.tensor_copy(gbc, gps)
            return gbc

        gk0 = gating(0)

        # ------------- TT expansion: matmuls + drains + dumps + gathers ----
        # MMs emitted nq-outer/i-inner: the 4 row-group tiles run concurrently
        # on the PE. Each bounce tile: 4 drains (DVE/ACT split), a contiguous
        # dump to DRAM (fast completion frees the bounce slot), then permuting
        # gather-loads (SWDGE on the idle GpSimd queue) into resident wb1/wb2.
        def expansion(w, l_sb, r_sb):
            for q in range(4):
                eps = {}
                for nq in range(4):
                    for i in range(4):
                        e = pp.tile([128, NCH], F32, tag="ps",
                                    name=f"pe{w}_{q}_{nq}_{i}")
                        nc.tensor.matmul(
                            e, l_sb[ds(32 * i, R), ts(q, 128)],
                            r_sb[ds(32 * i, R), ts(nq, NCH)],
                            tile_position=(32 * i, 0),
                        )
                        eps[(nq, i)] = e
                for i in range(4):
                    mt = 4 * q + i
                    bt = bounce.tile([128, 2048], BF16, tag="bt",
                                     name=f"bt{w}_{q}_{i}")
                    for nq in range(4):
                        if (nq + i) % 2 == 0:
                            nc.vector.tensor_copy(bt[:, ts(nq, NCH)], eps[(nq, i)])
                        else:
                            nc.scalar.activation(
                                bt[:, ts(nq, NCH)], eps[(nq, i)], AF.Copy)
                    slot = (w - 1) * 16 + mt
                    eng = nc.sync if i % 2 == 0 else nc.scalar
                    eng.dma_start(wdump[ds(slot, 1)].squeeze(), bt)
                    if w == 1:
                        for a2 in range(2):
                            sv = mt // 2
                            s_ap = wdump[ds(slot, 1)].squeeze().rearrange(
                                "(a2 x) (b y) -> a2 b x y", a2=2, y=64
                            )[ds(a2, 1)].squeeze()
                            d_ap = wb1[ds(64 * (mt % 2) + 32 * a2, 32), sv] \
                                .rearrange("b (x y) -> b x y", y=64)
                            nc.gpsimd.dma_start(d_ap, s_ap)
                    else:
                        for fl in range(4):
                            f1 = 4 * mt + fl
                            kcv, fhv = f1 // 2, f1 % 2
                            s_ap = wdump[ds(slot, 1)].squeeze().rearrange(
                                "(fl i1) (f2 i2) -> fl f2 i1 i2", fl=4, i2=32
                            )[ds(fl, 1)].squeeze()
                            d_ap = wb2[ds(64 * fhv, 64), kcv].rearrange(
                                "f2 (i1 i2) -> f2 i1 i2", i2=32)
                            nc.gpsimd.dma_start(d_ap, s_ap)

        expansion(1, g1_sb, g2_sb)
        expansion(2, c1_sb, c2_sb)

        # ---------------- ffn1 ----------------
        def ffn1(nch):
            xt_t = xts[nch]
            for grp in range(4):
                ps_l1 = [
                    pp.tile([128, NCH], F32, tag="ps", name=f"ps1_{nch}_{grp}_{i}")
                    for i in range(8)
                ]
                for s in range(8):
                    for j in range(8):
                        m = grp * 8 + j
                        nc.tensor.matmul(
                            ps_l1[j], wb1[:, s, ts(m, 128)], xt_t[:, s],
                            start=(s == 0), stop=(s == 7),
                        )
                for j in range(8):
                    nc.scalar.activation(
                        ht[:, grp * 8 + j], ps_l1[j], AF.Gelu_apprx_tanh
                    )

        ffn1(0)

        # ---------------- ffn2 + scale + RS per chunk ----------------
        def ffn2(nch, gbc):
            ps_l = [
                pp.tile([128, NCH], F32, tag="ps", name=f"ps2_{nch}_{i}")
                for i in range(8)
            ]
            for kc in range(32):
                for m2 in range(8):
                    nc.tensor.matmul(
                        ps_l[m2], wb2[:, kc, ts(m2, 128)], ht[:, kc],
                        start=(kc == 0), stop=(kc == 31),
                    )
            ob = obp.tile([128, 8, NCH], BF16, tag="ob", name=f"ob_{nch}")
            for m2 in range(8):
                nc.vector.tensor_mul(ob[:, m2], ps_l[m2], gbc)
                nc.vector.tensor_scalar_mul(ob[:, m2], ob[:, m2], pw_sb[:, ds(m2, 1)])
            nc.scalar.dma_start(
                cc_in[nch].ap().rearrange("(m2 p) n -> p m2 n", p=128), ob
            )
            nc.gpsimd.collective_compute(
                "ReduceScatter",
                mybir.AluOpType.add,
                replica_groups=GROUPS,
                ins=[cc_in[nch][:]],
                outs=[cc_out[nch][:]],
            )
            nc.scalar.dma_start(opiece[:, ts(nch, NCH)], cc_out[nch][:])

        ffn2(0, gk0)

        for nch in range(1, NNCH):
            load_xt(nch)
            gbc = gating(nch)
            ffn1(nch)
            ffn2(nch, gbc)


def build(verbose=False):
    nc = bacc.Bacc("TRN2", target_bir_lowering=False, debug=False, num_devices=NCORES)
    with tile.TileContext(nc) as tc:
        _emit(nc, tc)
    nc.compile()
    return nc


def make_in_maps(inputs):
    tokens = inputs["tokens"]
    bf = ml_dtypes.bfloat16
    in_maps = []
    for c in range(NCORES):
        half, k = c // 4, c % 4
        xt = np.ascontiguousarray(
            tokens[half * NHALF:(half + 1) * NHALF].T
        ).astype(bf)
        g1t = inputs["ffn1_core1"][k].transpose(2, 0, 1).reshape(R, IN1 * F1)
        g2 = inputs["ffn1_core2"][k].reshape(R, IN2 * F2)
        c1t = inputs["ffn2_core1"][k].transpose(2, 0, 1).reshape(R, F1 * IN1)
        c2 = inputs["ffn2_core2"][k].reshape(R, F2 * IN2)

        def pack_lhs(m):  # [R, 2048] -> [128, 512]: row group i gets chunk 4q+i
            out = np.zeros((128, 512), np.float32)
            for q in range(4):
                for i in range(4):
                    out[32 * i:32 * i + R, 128 * q:128 * (q + 1)] = \
                        m[:, 128 * (4 * q + i):128 * (4 * q + i + 1)]
            return out

        def pack_rhs(m):  # [R, 2048] -> [128, 2048]: replicate per row group
            out = np.zeros((128, 2048), np.float32)
            for i in range(4):
                out[32 * i:32 * i + R] = m
            return out
        pbt = np.ascontiguousarray(inputs["path_bases"].T).astype(bf)
        pwk = np.ascontiguousarray(
            (1.0 + inputs["path_weights"][k]).reshape(8, 128).T
        ).astype(np.float32)
        selk = np.zeros((K, 1), np.float32)
        selk[k, 0] = 1.0
        in_maps.append({
            "xT": xt,
            "g1pk": pack_lhs(g1t).astype(bf), "g2pk": pack_rhs(g2).astype(bf),
            "c1pk": pack_lhs(c1t).astype(bf), "c2pk": pack_rhs(c2).astype(bf),
            "pbT": pbt, "pw": pwk, "sel": selk,
            "ones4": np.ones((K, 1), np.float32),
            "ones1": np.ones((1, 128), np.float32),
        })
    return in_maps


def assemble(results):
    out = np.empty((NTOK, D), np.float32)
    for c in range(NCORES):
        half, k = c // 4, c % 4
        piece = results[c]["opiece"]  # [256 d-slice, 1024 tokens] bf16
        out[half * NHALF:(half + 1) * NHALF, k * 256:(k + 1) * 256] = \
            piece.T.astype(np.float32)
    return out


_NC = None


def run(inputs, trace=False):
    global _NC
    if _NC is None:
        _NC = build()
    res = run_bass_kernel_spmd(
        _NC, make_in_maps(inputs), core_ids=list(range(NCORES)), trace=trace
    )
    return assemble(res.results), res


def kernel(**inputs):
    out, _ = run(inputs)
    return out


# revision 37
# speedup vs baseline: 1.0432x; 1.0151x over previous
"""SuperposedExpert (K TT-factorized FFN paths + holographic routing) on 8 trn2 cores.

Strategy: expert x data parallel. Core c handles path k = c % 4 for token half
c // 4. On-device per core:
  1. logits/softmax gating from bf16 tokens (tiny matmuls on PE), per n-chunk.
  2. TT expansion: W = G1 x_r G2 via rank-16 row-packed matmuls; PSUM drained
     (DVE/ACT split) to bounce tiles, then permuting SBUF->SBUF DMA scatters
     convert the Kronecker-mixed layout [(a,x),(b,y)] into the dense matmul
     layout [(a,b),(x,y)] directly into RESIDENT weight tiles (wb1, wb2).
     No DRAM roundtrip. W1 scatters issue on sync (HWDGE), W2 on gpsimd (SWDGE)
     to spread sequencer issue cost across idle engines.
  3. Dense bf16 FFN per 512-token chunk: hT = gelu(W1^T @ xT), oT = W2^T @ hT.
  4. Scale by gate[n] (partition-broadcast) and (1 + path_weight[d]), cast bf16,
     ReduceScatter(add) over the 4 cores sharing the token half, per chunk.
Host only reshapes/casts inputs and concatenates/transposes the output pieces.
"""

import numpy as np
import ml_dtypes

import concourse.bass as bass
import concourse.tile as tile
from concourse import bacc, mybir
from concourse.bass import ds, ts
from concourse.bass_utils import run_bass_kernel_spmd

BF16 = mybir.dt.bfloat16
F32 = mybir.dt.float32
AF = mybir.ActivationFunctionType

K = 4
D = 1024            # d_model
DFF = 4096          # d_ff
R = 16              # tt rank
IN1, IN2 = 32, 32   # d_model = IN1 * IN2   (a, b)
F1, F2 = 64, 64     # d_ff    = F1 * F2     (x, y)
NTOK = 2048
NCORES = 8
NHALF = NTOK // 2   # tokens per core
NCH = 512           # n-chunk (psum bank = 512 fp32)
NNCH = NHALF // NCH
GROUPS = [[0, 1, 2, 3], [4, 5, 6, 7]]


def _emit(nc, tc):
    # ---------------- I/O ----------------
    xT = nc.dram_tensor("xT", [D, NHALF], BF16, kind="ExternalInput")
    # TT cores packed for 4-way row-tiled rank-16 matmuls: row group i
    # (partitions 32i..32i+15) holds stationary chunk 4q+i / a replica of
    # the moving operand.
    g1pk = nc.dram_tensor("g1pk", [128, 512], BF16, kind="ExternalInput")
    g2pk = nc.dram_tensor("g2pk", [128, 2048], BF16, kind="ExternalInput")
    c1pk = nc.dram_tensor("c1pk", [128, 512], BF16, kind="ExternalInput")
    c2pk = nc.dram_tensor("c2pk", [128, 2048], BF16, kind="ExternalInput")
    pbT = nc.dram_tensor("pbT", [D, K], BF16, kind="ExternalInput")
    pw = nc.dram_tensor("pw", [128, 8], F32, kind="ExternalInput")
    sel = nc.dram_tensor("sel", [K, 1], F32, kind="ExternalInput")
    ones4 = nc.dram_tensor("ones4", [K, 1], F32, kind="ExternalInput")
    ones1 = nc.dram_tensor("ones1", [1, 128], F32, kind="ExternalInput")
    opiece = nc.dram_tensor("opiece", [D // K, NHALF], BF16, kind="ExternalOutput")

    # DRAM bounce buffers for the per-chunk ReduceScatter (bf16)
    cc_in = [nc.dram_tensor(f"cc_in{i}", [D, NCH], BF16) for i in range(NNCH)]
    cc_out = [nc.dram_tensor(f"cc_out{i}", [D // K, NCH], BF16) for i in range(NNCH)]
    # DRAM staging for the W permute: bounce tiles dumped verbatim (big
    # contiguous descriptors, fast completion), permuting happens on the
    # load side (DRAM src APs are unrestricted; SBUF dst partition-first).
    wdump = nc.dram_tensor("wdump", [32, 128, 2048], BF16)

    with (
        tc.tile_pool(name="big", bufs=1) as big,
        tc.tile_pool(name="small", bufs=1) as small,
        tc.tile_pool(name="xtp", bufs=1) as xtp,
        tc.tile_pool(name="gkp", bufs=2) as gkp,
        tc.tile_pool(name="bounce", bufs=2) as bounce,
        tc.tile_pool(name="obp", bufs=1) as obp,
        tc.tile_pool(name="pp", bufs=8, space="PSUM") as pp,
    ):
        # ---------------- resident weight tiles ----------------
        # W1 dense layout: [p=(a%4, b), s=d-chunk, f=(x,y)]
        wb1 = big.tile([128, 8, DFF], BF16, tag="wb1")
        # W2 dense layout: [p=(f1%2, f2), kc=f-chunk, d=(i1,i2)]
        wb2 = big.tile([128, 32, D], BF16, tag="wb2")
        ht = big.tile([128, 32, NCH], BF16, tag="ht")  # single buffer, serial reuse

        # ---------------- small loads ----------------
        pbt_sb = small.tile([128, 8, K], BF16, tag="pbt")
        nc.sync.dma_start(pbt_sb, pbT.ap().rearrange("(t p) k -> p t k", p=128))
        pw_sb = small.tile([128, 8], F32, tag="pw")
        nc.sync.dma_start(pw_sb, pw.ap())
        sel_sb = small.tile([K, 1], F32, tag="sel")
        nc.sync.dma_start(sel_sb, sel.ap())
        ones4_sb = small.tile([K, 1], F32, tag="ones4")
        nc.sync.dma_start(ones4_sb, ones4.ap())
        ones1_sb = small.tile([1, 128], F32, tag="ones1")
        nc.sync.dma_start(ones1_sb, ones1.ap())

        g1_sb = small.tile([128, 512], BF16, tag="g1")
        nc.sync.dma_start(g1_sb, g1pk.ap())
        g2_sb = small.tile([128, 2048], BF16, tag="g2")
        nc.sync.dma_start(g2_sb, g2pk.ap())
        c1_sb = small.tile([128, 512], BF16, tag="c1")
        nc.sync.dma_start(c1_sb, c1pk.ap())
        c2_sb = small.tile([128, 2048], BF16, tag="c2")
        nc.sync.dma_start(c2_sb, c2pk.ap())

        xt_view = xT.ap().rearrange("(t p) n -> p t n", p=128)
        xts = {}

        def load_xt(nch):
            xt_t = xtp.tile([128, 8, NCH], BF16, tag="xt", name=f"xt_{nch}")
            nc.sync.dma_start(xt_t, xt_view[:, :, ts(nch, NCH)])
            xts[nch] = xt_t

        load_xt(0)

        # ---------------- gating helper (per n-chunk) ----------------
        def gating(nch):
            xt_t = xts[nch]
            expl = gkp.tile([K, NCH], F32, tag="expl", name=f"expl_{nch}")
            lps = pp.tile([K, NCH], F32, tag="ps", name=f"lps_{nch}")
            for t in range(8):
                nc.tensor.matmul(
                    lps, pbt_sb[:, t], xt_t[:, t], start=(t == 0), stop=(t == 7)
                )
            nc.scalar.activation(expl, lps, AF.Exp)
            den = pp.tile([1, NCH], F32, tag="ps", name=f"den_{nch}")
            num = pp.tile([1, NCH], F32, tag="ps", name=f"num_{nch}")
            nc.tensor.matmul(den, ones4_sb, expl)
            nc.tensor.matmul(num, sel_sb, expl)
            gk = gkp.tile([1, NCH], F32, tag="gk", name=f"gk_{nch}")
            nc.vector.reciprocal(gk, den)
            nc.vector.tensor_mul(gk, num, gk)
            # broadcast gate row to 128 partitions via PE (DVE can't
            # partition-broadcast): gbc = ones1^T @ gk
            gbc = gkp.tile([128, NCH], BF16, tag="gbc", name=f"gbc_{nch}")
            gps = pp.tile([128, NCH], F32, tag="ps", name=f"gps_{nch}")
            nc.tensor.matmul(gps, ones1_sb, gk)
            nc.vector.tensor_copy(gbc, gps)
            return gbc

        gk0 = gating(0)

        # ------------- TT expansion: matmuls + drains + dumps + gathers ----
        # MMs emitted nq-outer/i-inner: the 4 row-group tiles run concurrently
        # on the PE. Each bounce tile: 4 drains (DVE/ACT split), a contiguous
        # dump to DRAM (fast completion frees the bounce slot), then permuting
        # gather-loads (SWDGE on the idle GpSimd queue) into resident wb1/wb2.
        def expansion(w, l_sb, r_sb):
            for q in range(4):
                eps = {}
                for nq in range(4):
                    for i in range(4):
                        e = pp.tile([128, NCH], F32, tag="ps",
                                    name=f"pe{w}_{q}_{nq}_{i}")
                        nc.tensor.matmul(
                            e, l_sb[ds(32 * i, R), ts(q, 128)],
                            r_sb[ds(32 * i, R), ts(nq, NCH)],
                            tile_position=(32 * i, 0),
                        )
                        eps[(nq, i)] = e
                for i in range(4):
                    mt = 4 * q + i
                    bt = bounce.tile([128, 2048], BF16, tag="bt",
                                     name=f"bt{w}_{q}_{i}")
                    for nq in range(4):
                        if (nq + i) % 2 == 0:
                            nc.vector.tensor_copy(bt[:, ts(nq, NCH)], eps[(nq, i)])
                        else:
                            nc.scalar.activation(
                                bt[:, ts(nq, NCH)], eps[(nq, i)], AF.Copy)
                    slot = (w - 1) * 16 + mt
                    eng = nc.sync if i % 2 == 0 else nc.scalar
                    eng.dma_start(wdump[ds(slot, 1)].squeeze(), bt)
                    if w == 1:
                        for a2 in range(2):
                            sv = mt // 2
                            s_ap = wdump[ds(slot, 1)].squeeze().rearrange(
                                "(a2 x) (b y) -> a2 b x y", a2=2, y=64
                            )[ds(a2, 1)].squeeze()
                            d_ap = wb1[ds(64 * (mt % 2) + 32 * a2, 32), sv] \
                                .rearrange("b (x y) -> b x y", y=64)
                            nc.gpsimd.dma_start(d_ap, s_ap)
                    else:
                        for fl in range(4):
                            f1 = 4 * mt + fl
                            kcv, fhv = f1 // 2, f1 % 2
                            s_ap = wdump[ds(slot, 1)].squeeze().rearrange(
                                "(fl i1) (f2 i2) -> fl f2 i1 i2", fl=4, i2=32
                            )[ds(fl, 1)].squeeze()
                            d_ap = wb2[ds(64 * fhv, 64), kcv].rearrange(
                                "f2 (i1 i2) -> f2 i1 i2", i2=32)
                            nc.gpsimd.dma_start(d_ap, s_ap)

        expansion(1, g1_sb, g2_sb)
        expansion(2, c1_sb, c2_sb)

        # ---------------- ffn1 ----------------
        def ffn1(nch):
            xt_t = xts[nch]
            for grp in range(4):
                ps_l1 = [
                    pp.tile([128, NCH], F32, tag="ps", name=f"ps1_{nch}_{grp}_{i}")
                    for i in range(8)
                ]
                for s in range(8):
                    for j in range(8):
                        m = grp * 8 + j
                        nc.tensor.matmul(
                            ps_l1[j], wb1[:, s, ts(m, 128)], xt_t[:, s],
                            start=(s == 0), stop=(s == 7),
                        )
                for j in range(8):
                    nc.scalar.activation(
                        ht[:, grp * 8 + j], ps_l1[j], AF.Gelu_apprx_tanh
                    )

        ffn1(0)

        # ---------------- ffn2 + scale + RS per chunk ----------------
        def ffn2(nch, gbc):
            ps_l = [
                pp.tile([128, NCH], F32, tag="ps", name=f"ps2_{nch}_{i}")
                for i in range(8)
            ]
            for kc in range(32):
                for m2 in range(8):
                    nc.tensor.matmul(
                        ps_l[m2], wb2[:, kc, ts(m2, 128)], ht[:, kc],
                        start=(kc == 0), stop=(kc == 31),
                    )
            ob = obp.tile([128, 8, NCH], BF16, tag="ob", name=f"ob_{nch}")
            for m2 in range(8):
                nc.vector.tensor_mul(ob[:, m2], ps_l[m2], gbc)
                nc.vector.tensor_scalar_mul(ob[:, m2], ob[:, m2], pw_sb[:, ds(m2, 1)])
            nc.scalar.dma_start(
                cc_in[nch].ap().rearrange("(m2 p) n -> p m2 n", p=128), ob
            )
            nc.gpsimd.collective_compute(
                "ReduceScatter",
                mybir.AluOpType.add,
                replica_groups=GROUPS,
                ins=[cc_in[nch][:]],
                outs=[cc_out[nch][:]],
            )
            # opiece waits on the RS — keep it off scalar so it can't
            # head-of-line-block the next chunk's gelus behind that wait
            nc.gpsimd.dma_start(opiece[:, ts(nch, NCH)], cc_out[nch][:])

        ffn2(0, gk0)

        for nch in range(1, NNCH):
            load_xt(nch)
            gbc = gating(nch)
            ffn1(nch)
            ffn2(nch, gbc)


def build(verbose=False):
    nc = bacc.Bacc("TRN2", target_bir_lowering=False, debug=False, num_devices=NCORES)
    with tile.TileContext(nc) as tc:
        _emit(nc, tc)
    nc.compile()
    return nc


def make_in_maps(inputs):
    tokens = inputs["tokens"]
    bf = ml_dtypes.bfloat16
    in_maps = []
    for c in range(NCORES):
        half, k = c // 4, c % 4
        xt = np.ascontiguousarray(
            tokens[half * NHALF:(half + 1) * NHALF].T
        ).astype(bf)
        g1t = inputs["ffn1_core1"][k].transpose(2, 0, 1).reshape(R, IN1 * F1)
        g2 = inputs["ffn1_core2"][k].reshape(R, IN2 * F2)
        c1t = inputs["ffn2_core1"][k].transpose(2, 0, 1).reshape(R, F1 * IN1)
        c2 = inputs["ffn2_core2"][k].reshape(R, F2 * IN2)

        def pack_lhs(m):  # [R, 2048] -> [128, 512]: row group i gets chunk 4q+i
            out = np.zeros((128, 512), np.float32)
            for q in range(4):
                for i in range(4):
                    out[32 * i:32 * i + R, 128 * q:128 * (q + 1)] = \
                        m[:, 128 * (4 * q + i):128 * (4 * q + i + 1)]
            return out

        def pack_rhs(m):  # [R, 2048] -> [128, 2048]: replicate per row group
            out = np.zeros((128, 2048), np.float32)
            for i in range(4):
                out[32 * i:32 * i + R] = m
            return out
        pbt = np.ascontiguousarray(inputs["path_bases"].T).astype(bf)
        pwk = np.ascontiguousarray(
            (1.0 + inputs["path_weights"][k]).reshape(8, 128).T
        ).astype(np.float32)
        selk = np.zeros((K, 1), np.float32)
        selk[k, 0] = 1.0
        in_maps.append({
            "xT": xt,
            "g1pk": pack_lhs(g1t).astype(bf), "g2pk": pack_rhs(g2).astype(bf),
            "c1pk": pack_lhs(c1t).astype(bf), "c2pk": pack_rhs(c2).astype(bf),
            "pbT": pbt, "pw": pwk, "sel": selk,
            "ones4": np.ones((K, 1), np.float32),
            "ones1": np.ones((1, 128), np.float32),
        })
    return in_maps


def assemble(results):
    out = np.empty((NTOK, D), np.float32)
    for c in range(NCORES):
        half, k = c // 4, c % 4
        piece = results[c]["opiece"]  # [256 d-slice, 1024 tokens] bf16
        out[half * NHALF:(half + 1) * NHALF, k * 256:(k + 1) * 256] = \
            piece.T.astype(np.float32)
    return out


_NC = None


def run(inputs, trace=False):
    global _NC
    if _NC is None:
        _NC = build()
    res = run_bass_kernel_spmd(
        _NC, make_in_maps(inputs), core_ids=list(range(NCORES)), trace=trace
    )
    return assemble(res.results), res


def kernel(**inputs):
    out, _ = run(inputs)
    return out


# revision 38
# speedup vs baseline: 1.0443x; 1.0010x over previous
"""SuperposedExpert (K TT-factorized FFN paths + holographic routing) on 8 trn2 cores.

Strategy: expert x data parallel. Core c handles path k = c % 4 for token half
c // 4. On-device per core:
  1. logits/softmax gating from bf16 tokens (tiny matmuls on PE), per n-chunk.
  2. TT expansion: W = G1 x_r G2 via rank-16 row-packed matmuls; PSUM drained
     (DVE/ACT split) to bounce tiles, then permuting SBUF->SBUF DMA scatters
     convert the Kronecker-mixed layout [(a,x),(b,y)] into the dense matmul
     layout [(a,b),(x,y)] directly into RESIDENT weight tiles (wb1, wb2).
     No DRAM roundtrip. W1 scatters issue on sync (HWDGE), W2 on gpsimd (SWDGE)
     to spread sequencer issue cost across idle engines.
  3. Dense bf16 FFN per 512-token chunk: hT = gelu(W1^T @ xT), oT = W2^T @ hT.
  4. Scale by gate[n] (partition-broadcast) and (1 + path_weight[d]), cast bf16,
     ReduceScatter(add) over the 4 cores sharing the token half, per chunk.
Host only reshapes/casts inputs and concatenates/transposes the output pieces.
"""

import numpy as np
import ml_dtypes

import concourse.bass as bass
import concourse.tile as tile
from concourse import bacc, mybir
from concourse.bass import ds, ts
from concourse.bass_utils import run_bass_kernel_spmd

BF16 = mybir.dt.bfloat16
F32 = mybir.dt.float32
AF = mybir.ActivationFunctionType

K = 4
D = 1024            # d_model
DFF = 4096          # d_ff
R = 16              # tt rank
IN1, IN2 = 32, 32   # d_model = IN1 * IN2   (a, b)
F1, F2 = 64, 64     # d_ff    = F1 * F2     (x, y)
NTOK = 2048
NCORES = 8
NHALF = NTOK // 2   # tokens per core
NCH = 512           # n-chunk (psum bank = 512 fp32)
NNCH = NHALF // NCH
GROUPS = [[0, 1, 2, 3], [4, 5, 6, 7]]


def _emit(nc, tc):
    # ---------------- I/O ----------------
    xT = nc.dram_tensor("xT", [D, NHALF], BF16, kind="ExternalInput")
    # TT cores packed for 4-way row-tiled rank-16 matmuls: row group i
    # (partitions 32i..32i+15) holds stationary chunk 4q+i / a replica of
    # the moving operand.
    g1pk = nc.dram_tensor("g1pk", [128, 512], BF16, kind="ExternalInput")
    g2pk = nc.dram_tensor("g2pk", [128, 2048], BF16, kind="ExternalInput")
    c1pk = nc.dram_tensor("c1pk", [128, 512], BF16, kind="ExternalInput")
    c2pk = nc.dram_tensor("c2pk", [128, 2048], BF16, kind="ExternalInput")
    pbT = nc.dram_tensor("pbT", [D, K], BF16, kind="ExternalInput")
    pw = nc.dram_tensor("pw", [128, 8], F32, kind="ExternalInput")
    sel = nc.dram_tensor("sel", [K, 1], F32, kind="ExternalInput")
    ones4 = nc.dram_tensor("ones4", [K, 1], F32, kind="ExternalInput")
    ones1 = nc.dram_tensor("ones1", [1, 128], F32, kind="ExternalInput")
    opiece = nc.dram_tensor("opiece", [D // K, NHALF], BF16, kind="ExternalOutput")

    # DRAM bounce buffers for the per-chunk ReduceScatter (bf16)
    cc_in = [nc.dram_tensor(f"cc_in{i}", [D, NCH], BF16) for i in range(NNCH)]
    cc_out = [nc.dram_tensor(f"cc_out{i}", [D // K, NCH], BF16) for i in range(NNCH)]
    # DRAM staging for the W permute: bounce tiles dumped verbatim (big
    # contiguous descriptors, fast completion), permuting happens on the
    # load side (DRAM src APs are unrestricted; SBUF dst partition-first).
    wdump = nc.dram_tensor("wdump", [32, 128, 2048], BF16)

    with (
        tc.tile_pool(name="big", bufs=1) as big,
        tc.tile_pool(name="small", bufs=1) as small,
        tc.tile_pool(name="xtp", bufs=1) as xtp,
        tc.tile_pool(name="gkp", bufs=2) as gkp,
        tc.tile_pool(name="bounce", bufs=2) as bounce,
        tc.tile_pool(name="obp", bufs=1) as obp,
        tc.tile_pool(name="pp", bufs=8, space="PSUM") as pp,
    ):
        # ---------------- resident weight tiles ----------------
        # W1 dense layout: [p=(a%4, b), s=d-chunk, f=(x,y)]
        wb1 = big.tile([128, 8, DFF], BF16, tag="wb1")
        # W2 dense layout: [p=(f1%2, f2), kc=f-chunk, d=(i1,i2)]
        wb2 = big.tile([128, 32, D], BF16, tag="wb2")
        ht = big.tile([128, 32, NCH], BF16, tag="ht")  # single buffer, serial reuse

        # ---------------- small loads ----------------
        # Expansion operands first: the TT-expansion matmuls are the first
        # long-pole consumers; gating inputs and xt0 follow behind them.
        g1_sb = small.tile([128, 512], BF16, tag="g1")
        nc.sync.dma_start(g1_sb, g1pk.ap())
        g2_sb = small.tile([128, 2048], BF16, tag="g2")
        nc.sync.dma_start(g2_sb, g2pk.ap())
        c1_sb = small.tile([128, 512], BF16, tag="c1")
        nc.sync.dma_start(c1_sb, c1pk.ap())
        c2_sb = small.tile([128, 2048], BF16, tag="c2")
        nc.sync.dma_start(c2_sb, c2pk.ap())

        pbt_sb = small.tile([128, 8, K], BF16, tag="pbt")
        nc.sync.dma_start(pbt_sb, pbT.ap().rearrange("(t p) k -> p t k", p=128))
        pw_sb = small.tile([128, 8], F32, tag="pw")
        nc.sync.dma_start(pw_sb, pw.ap())
        sel_sb = small.tile([K, 1], F32, tag="sel")
        nc.sync.dma_start(sel_sb, sel.ap())
        ones4_sb = small.tile([K, 1], F32, tag="ones4")
        nc.sync.dma_start(ones4_sb, ones4.ap())
        ones1_sb = small.tile([1, 128], F32, tag="ones1")
        nc.sync.dma_start(ones1_sb, ones1.ap())

        xt_view = xT.ap().rearrange("(t p) n -> p t n", p=128)
        xts = {}

        def load_xt(nch):
            xt_t = xtp.tile([128, 8, NCH], BF16, tag="xt", name=f"xt_{nch}")
            nc.sync.dma_start(xt_t, xt_view[:, :, ts(nch, NCH)])
            xts[nch] = xt_t

        load_xt(0)

        # ---------------- gating helper (per n-chunk) ----------------
        def gating(nch):
            xt_t = xts[nch]
            expl = gkp.tile([K, NCH], F32, tag="expl", name=f"expl_{nch}")
            lps = pp.tile([K, NCH], F32, tag="ps", name=f"lps_{nch}")
            for t in range(8):
                nc.tensor.matmul(
                    lps, pbt_sb[:, t], xt_t[:, t], start=(t == 0), stop=(t == 7)
                )
            nc.scalar.activation(expl, lps, AF.Exp)
            den = pp.tile([1, NCH], F32, tag="ps", name=f"den_{nch}")
            num = pp.tile([1, NCH], F32, tag="ps", name=f"num_{nch}")
            nc.tensor.matmul(den, ones4_sb, expl)
            nc.tensor.matmul(num, sel_sb, expl)
            gk = gkp.tile([1, NCH], F32, tag="gk", name=f"gk_{nch}")
            nc.vector.reciprocal(gk, den)
            nc.vector.tensor_mul(gk, num, gk)
            # broadcast gate row to 128 partitions via PE (DVE can't
            # partition-broadcast): gbc = ones1^T @ gk
            gbc = gkp.tile([128, NCH], BF16, tag="gbc", name=f"gbc_{nch}")
            gps = pp.tile([128, NCH], F32, tag="ps", name=f"gps_{nch}")
            nc.tensor.matmul(gps, ones1_sb, gk)
            nc.vector.tensor_copy(gbc, gps)
            return gbc

        gk0 = gating(0)

        # ------------- TT expansion: matmuls + drains + dumps + gathers ----
        # MMs emitted nq-outer/i-inner: the 4 row-group tiles run concurrently
        # on the PE. Each bounce tile: 4 drains (DVE/ACT split), a contiguous
        # dump to DRAM (fast completion frees the bounce slot), then permuting
        # gather-loads (SWDGE on the idle GpSimd queue) into resident wb1/wb2.
        def expansion(w, l_sb, r_sb):
            for q in range(4):
                eps = {}
                for nq in range(4):
                    for i in range(4):
                        e = pp.tile([128, NCH], F32, tag="ps",
                                    name=f"pe{w}_{q}_{nq}_{i}")
                        nc.tensor.matmul(
                            e, l_sb[ds(32 * i, R), ts(q, 128)],
                            r_sb[ds(32 * i, R), ts(nq, NCH)],
                            tile_position=(32 * i, 0),
                        )
                        eps[(nq, i)] = e
                for i in range(4):
                    mt = 4 * q + i
                    bt = bounce.tile([128, 2048], BF16, tag="bt",
                                     name=f"bt{w}_{q}_{i}")
                    for nq in range(4):
                        if (nq + i) % 2 == 0:
                            nc.vector.tensor_copy(bt[:, ts(nq, NCH)], eps[(nq, i)])
                        else:
                            nc.scalar.activation(
                                bt[:, ts(nq, NCH)], eps[(nq, i)], AF.Copy)
                    slot = (w - 1) * 16 + mt
                    eng = nc.sync if i % 2 == 0 else nc.scalar
                    eng.dma_start(wdump[ds(slot, 1)].squeeze(), bt)
                    if w == 1:
                        for a2 in range(2):
                            sv = mt // 2
                            s_ap = wdump[ds(slot, 1)].squeeze().rearrange(
                                "(a2 x) (b y) -> a2 b x y", a2=2, y=64
                            )[ds(a2, 1)].squeeze()
                            d_ap = wb1[ds(64 * (mt % 2) + 32 * a2, 32), sv] \
                                .rearrange("b (x y) -> b x y", y=64)
                            nc.gpsimd.dma_start(d_ap, s_ap)
                    else:
                        for fl in range(4):
                            f1 = 4 * mt + fl
                            kcv, fhv = f1 // 2, f1 % 2
                            s_ap = wdump[ds(slot, 1)].squeeze().rearrange(
                                "(fl i1) (f2 i2) -> fl f2 i1 i2", fl=4, i2=32
                            )[ds(fl, 1)].squeeze()
                            d_ap = wb2[ds(64 * fhv, 64), kcv].rearrange(
                                "f2 (i1 i2) -> f2 i1 i2", i2=32)
                            nc.gpsimd.dma_start(d_ap, s_ap)

        expansion(1, g1_sb, g2_sb)
        expansion(2, c1_sb, c2_sb)

        # ---------------- ffn1 ----------------
        def ffn1(nch):
            xt_t = xts[nch]
            for grp in range(4):
                ps_l1 = [
                    pp.tile([128, NCH], F32, tag="ps", name=f"ps1_{nch}_{grp}_{i}")
                    for i in range(8)
                ]
                for s in range(8):
                    for j in range(8):
                        m = grp * 8 + j
                        nc.tensor.matmul(
                            ps_l1[j], wb1[:, s, ts(m, 128)], xt_t[:, s],
                            start=(s == 0), stop=(s == 7),
                        )
                for j in range(8):
                    nc.scalar.activation(
                        ht[:, grp * 8 + j], ps_l1[j], AF.Gelu_apprx_tanh
                    )

        ffn1(0)

        # ---------------- ffn2 + scale + RS per chunk ----------------
        def ffn2(nch, gbc):
            ps_l = [
                pp.tile([128, NCH], F32, tag="ps", name=f"ps2_{nch}_{i}")
                for i in range(8)
            ]
            for kc in range(32):
                for m2 in range(8):
                    nc.tensor.matmul(
                        ps_l[m2], wb2[:, kc, ts(m2, 128)], ht[:, kc],
                        start=(kc == 0), stop=(kc == 31),
                    )
            ob = obp.tile([128, 8, NCH], BF16, tag="ob", name=f"ob_{nch}")
            for m2 in range(8):
                nc.vector.tensor_mul(ob[:, m2], ps_l[m2], gbc)
                nc.vector.tensor_scalar_mul(ob[:, m2], ob[:, m2], pw_sb[:, ds(m2, 1)])
            nc.scalar.dma_start(
                cc_in[nch].ap().rearrange("(m2 p) n -> p m2 n", p=128), ob
            )
            nc.gpsimd.collective_compute(
                "ReduceScatter",
                mybir.AluOpType.add,
                replica_groups=GROUPS,
                ins=[cc_in[nch][:]],
                outs=[cc_out[nch][:]],
            )
            # opiece waits on the RS — keep it off scalar so it can't
            # head-of-line-block the next chunk's gelus behind that wait
            nc.gpsimd.dma_start(opiece[:, ts(nch, NCH)], cc_out[nch][:])

        ffn2(0, gk0)

        for nch in range(1, NNCH):
            load_xt(nch)
            gbc = gating(nch)
            ffn1(nch)
            ffn2(nch, gbc)


def build(verbose=False):
    nc = bacc.Bacc("TRN2", target_bir_lowering=False, debug=False, num_devices=NCORES)
    with tile.TileContext(nc) as tc:
        _emit(nc, tc)
    nc.compile()
    return nc


def make_in_maps(inputs):
    tokens = inputs["tokens"]
    bf = ml_dtypes.bfloat16
    in_maps = []
    for c in range(NCORES):
        half, k = c // 4, c % 4
        xt = np.ascontiguousarray(
            tokens[half * NHALF:(half + 1) * NHALF].T
        ).astype(bf)
        g1t = inputs["ffn1_core1"][k].transpose(2, 0, 1).reshape(R, IN1 * F1)
        g2 = inputs["ffn1_core2"][k].reshape(R, IN2 * F2)
        c1t = inputs["ffn2_core1"][k].transpose(2, 0, 1).reshape(R, F1 * IN1)
        c2 = inputs["ffn2_core2"][k].reshape(R, F2 * IN2)

        def pack_lhs(m):  # [R, 2048] -> [128, 512]: row group i gets chunk 4q+i
            out = np.zeros((128, 512), np.float32)
            for q in range(4):
                for i in range(4):
                    out[32 * i:32 * i + R, 128 * q:128 * (q + 1)] = \
                        m[:, 128 * (4 * q + i):128 * (4 * q + i + 1)]
            return out

        def pack_rhs(m):  # [R, 2048] -> [128, 2048]: replicate per row group
            out = np.zeros((128, 2048), np.float32)
            for i in range(4):
                out[32 * i:32 * i + R] = m
            return out
        pbt = np.ascontiguousarray(inputs["path_bases"].T).astype(bf)
        pwk = np.ascontiguousarray(
            (1.0 + inputs["path_weights"][k]).reshape(8, 128).T
        ).astype(np.float32)
        selk = np.zeros((K, 1), np.float32)
        selk[k, 0] = 1.0
        in_maps.append({
            "xT": xt,
            "g1pk": pack_lhs(g1t).astype(bf), "g2pk": pack_rhs(g2).astype(bf),
            "c1pk": pack_lhs(c1t).astype(bf), "c2pk": pack_rhs(c2).astype(bf),
            "pbT": pbt, "pw": pwk, "sel": selk,
            "ones4": np.ones((K, 1), np.float32),
            "ones1": np.ones((1, 128), np.float32),
        })
    return in_maps


def assemble(results):
    out = np.empty((NTOK, D), np.float32)
    for c in range(NCORES):
        half, k = c // 4, c % 4
        piece = results[c]["opiece"]  # [256 d-slice, 1024 tokens] bf16
        out[half * NHALF:(half + 1) * NHALF, k * 256:(k + 1) * 256] = \
            piece.T.astype(np.float32)
    return out


_NC = None


def run(inputs, trace=False):
    global _NC
    if _NC is None:
        _NC = build()
    res = run_bass_kernel_spmd(
        _NC, make_in_maps(inputs), core_ids=list(range(NCORES)), trace=trace
    )
    return assemble(res.results), res


def kernel(**inputs):
    out, _ = run(inputs)
    return out
